# revision 1
# baseline (speedup 1.0000x reference)
"""LLaDA2 MoE decoder layer on 8 TRN2 NeuronCores.

Token-sharded attention (each core: all 16 heads for its 128 tokens, kv
projection replicated), one AllGather of post-attention normed hidden
(transposed layout), expert-parallel dense MoE (2 experts/core, gate
columns permuted per-core so local experts are columns 0,1), shared
expert token-sharded. Host sums the 8 partial outputs.
"""
import numpy as np
import concourse.bass as bass
import concourse.bacc as bacc
import concourse.mybir as mybir
import concourse.tile as tile
from concourse.bass_utils import run_bass_kernel_spmd

AF = mybir.ActivationFunctionType
ALU = mybir.AluOpType
F32 = mybir.dt.float32
F32R = mybir.dt.float32r
BF16 = mybir.dt.bfloat16

B, S, H = 1, 1024, 2048
NH, HD, NKV, ROT = 16, 128, 4, 64
E, TOPK, G = 16, 4, 2
MI = 1024
T = S
NCORES = 8
TL = T // NCORES
SCAL = HD ** -0.5
EPS = 1e-6
NDH = H // 128
NMI = MI // 128

_BUILT = {}


def _spec():
    return [
        ("hidT", [H, T], F32), ("hidTl", [H, TL], F32),
        ("onec", [128, 1], F32R),
        ("cosl", [ROT, TL], F32), ("sinl", [ROT, TL], F32),
        ("cosf", [ROT, T], F32), ("sinf", [ROT, T], F32),
        ("qln", [HD, 1], F32), ("kln", [HD, 1], F32),
        ("ln1c", [H, 1], F32), ("ln2c", [H, 1], F32),
        ("wqT", [H, NH * HD], BF16), ("wkT", [H, NKV * HD], BF16),
        ("wvT", [H, NKV * HD], BF16), ("wdT", [NH * HD, H], BF16),
        ("gT", [H, E], F32R), ("eb", [1, E], F32),
        ("g0T", [H, MI], BF16), ("u0T", [H, MI], BF16), ("d0T", [MI, H], BF16),
        ("g1T", [H, MI], BF16), ("u1T", [H, MI], BF16), ("d1T", [MI, H], BF16),
        ("sgT", [H, MI], BF16), ("suT", [H, MI], BF16), ("sdT", [MI, H], BF16),
    ]


def _build():
    if "nc" in _BUILT:
        return _BUILT["nc"]
    nc = bacc.Bacc("TRN2", target_bir_lowering=False, debug=False,
                   num_devices=NCORES)
    I = {}
    for name, shp, dt in _spec():
        I[name] = nc.dram_tensor(name, shp, dt, kind="ExternalInput")
    routed = nc.dram_tensor("routed", [T, H], F32, kind="ExternalOutput")
    own = nc.dram_tensor("own", [TL, H], F32, kind="ExternalOutput")
    xout = nc.dram_tensor("xout", [H, TL], F32, kind="ExternalOutput")

    with tile.TileContext(nc) as tc, \
         tc.tile_pool(name="cst", bufs=1) as cst, \
         tc.tile_pool(name="big", bufs=16) as big, \
         tc.tile_pool(name="kro", bufs=4) as krop, \
         tc.tile_pool(name="vp", bufs=8) as vp, \
         tc.tile_pool(name="otp", bufs=16) as otp, \
         tc.tile_pool(name="agl", bufs=16) as agl, \
         tc.tile_pool(name="a12", bufs=16) as a12p, \
         tc.tile_pool(name="wrk", bufs=2) as wrk, \
         tc.tile_pool(name="w128", bufs=8) as w128, \
         tc.tile_pool(name="w512", bufs=2) as w512, \
         tc.tile_pool(name="yp", bufs=2) as yp, \
         tc.tile_pool(name="psA", bufs=4, space="PSUM") as psA, \
         tc.tile_pool(name="psB", bufs=4, space="PSUM") as psB, \
         tc.tile_pool(name="dram", bufs=1, space="DRAM") as dpool:

        ones = cst.tile([128, 1], F32R, tag="ones")
        nc.sync.dma_start(out=ones[:, :], in_=I["onec"][:, :])
        ones_bf = cst.tile([128, 1], BF16, tag="ones_bf")
        nc.vector.memset(ones_bf[:, :], 1.0)
        epsA = cst.tile([128, 1], F32, tag="epsA")
        nc.vector.memset(epsA[:, :], EPS)
        invH = cst.tile([128, 1], F32, tag="invH")
        nc.vector.memset(invH[:, :], 1.0 / H)
        invHD = cst.tile([128, 1], F32, tag="invHD")
        nc.vector.memset(invHD[:, :], 1.0 / HD)
        scalA = cst.tile([128, 1], F32, tag="scalA")
        nc.vector.memset(scalA[:, :], SCAL)

        def cload(name, shp, key):
            t_ = cst.tile(shp, F32, tag=key)
            nc.sync.dma_start(out=t_[:, :], in_=I[name][:, :])
            return t_
        qln = cload("qln", [HD, 1], "qln")
        kln = cload("kln", [HD, 1], "kln")
        cosl = cload("cosl", [ROT, TL], "cosl")
        sinl = cload("sinl", [ROT, TL], "sinl")
        cosf = cload("cosf", [ROT, T], "cosf")
        sinf = cload("sinf", [ROT, T], "sinf")
        ebbc = cst.tile([128, E], F32, tag="ebbc")
        nc.sync.dma_start(out=ebbc[:, :],
                          in_=I["eb"][0:1, :].partition_broadcast(128))

        def bcast(row_ap, n, tag, out_tile):
            d_ = dpool.tile([1, n], F32, tag=tag + "_d", bufs=2,
                            name=tag + "_d")
            nc.sync.dma_start(out=d_[0:1, :], in_=row_ap)
            nc.sync.dma_start(out=out_tile[:, :],
                              in_=d_[0:1, :].partition_broadcast(128))

        # ---- r_row over H from hidT (streamed) ----
        ssq = [psB.tile([1, 512], F32, tag="psB", name=f"ssq{c}")
               for c in range(2)]
        for i in range(NDH):
            ht = wrk.tile([128, T], F32, tag="hidT", bufs=2)
            nc.sync.dma_start(out=ht[:, :], in_=I["hidT"][i * 128:(i + 1) * 128, :])
            sq = wrk.tile([128, T], F32R, tag="sq", bufs=2)
            nc.scalar.activation(sq[:, :], ht[:, :], AF.Square)
            for c in range(2):
                nc.tensor.matmul(ssq[c][:, :], ones[:, :],
                                 sq[:, c * 512:(c + 1) * 512],
                                 start=(i == 0), stop=(i == NDH - 1))
        r_row = wrk.tile([1, T], F32, tag="rrow", bufs=1)
        rsq = wrk.tile([1, T], F32, tag="rsq", bufs=1)
        for c in range(2):
            nc.scalar.activation(rsq[0:1, c * 512:(c + 1) * 512], ssq[c][:, :],
                                 AF.Sqrt, bias=epsA[0:1, 0:1],
                                 scale=invH[0:1, 0:1])
        nc.vector.reciprocal(r_row[0:1, :], rsq[0:1, :])
        rbc = wrk.tile([128, T], F32, tag="rbc", bufs=1)
        bcast(r_row[0:1, :], T, "rbc", rbc)

        # ---- xnT = hidT * ln1 * r (transposed normed hidden, f32r) ----
        xnT = []
        for i in range(NDH):
            ht = wrk.tile([128, T], F32, tag="hidT", bufs=2)
            nc.sync.dma_start(out=ht[:, :], in_=I["hidT"][i * 128:(i + 1) * 128, :])
            lnc = wrk.tile([128, 1], F32, tag="lnc", bufs=2)
            nc.sync.dma_start(out=lnc[:, :], in_=I["ln1c"][i * 128:(i + 1) * 128, :])
            xt = big.tile([128, T], BF16, tag="big")
            nc.vector.scalar_tensor_tensor(xt[:, :], ht[:, :], lnc[:, 0:1],
                                           rbc[:, :], ALU.mult, ALU.mult)
            xnT.append(xt)

        # ---- local-token normed tiles for q projection ----
        ssl = psB.tile([1, TL], F32, tag="psB", name="ssl")
        for i in range(NDH):
            htl = wrk.tile([128, TL], F32, tag="htl", bufs=2)
            nc.sync.dma_start(out=htl[:, :], in_=I["hidTl"][i * 128:(i + 1) * 128, :])
            sql = wrk.tile([128, TL], F32R, tag="sql", bufs=2)
            nc.scalar.activation(sql[:, :], htl[:, :], AF.Square)
            nc.tensor.matmul(ssl[:, :], ones[:, :], sql[:, :],
                             start=(i == 0), stop=(i == NDH - 1))
        rls = wrk.tile([1, TL], F32, tag="rls", bufs=1)
        nc.scalar.activation(rls[0:1, :], ssl[:, :], AF.Sqrt,
                             bias=epsA[0:1, 0:1], scale=invH[0:1, 0:1])
        rl = wrk.tile([1, TL], F32, tag="rl", bufs=1)
        nc.vector.reciprocal(rl[0:1, :], rls[0:1, :])
        rlb = wrk.tile([128, TL], F32, tag="rlb", bufs=1)
        bcast(rl[0:1, :], TL, "rlb", rlb)
        xnTl = []
        for i in range(NDH):
            htl2 = wrk.tile([128, TL], F32, tag="htl", bufs=2)
            nc.sync.dma_start(out=htl2[:, :],
                              in_=I["hidTl"][i * 128:(i + 1) * 128, :])
            lnc2 = wrk.tile([128, 1], F32, tag="lnc", bufs=2)
            nc.sync.dma_start(out=lnc2[:, :], in_=I["ln1c"][i * 128:(i + 1) * 128, :])
            xl = wrk.tile([128, TL], BF16, tag="xnTl", bufs=16)
            nc.vector.scalar_tensor_tensor(xl[:, :], htl2[:, :], lnc2[:, 0:1],
                                           rlb[:, :], ALU.mult, ALU.mult)
            xnTl.append(xl)

        def rms_cols(ps, n, lnw, out_ap):
            """out = ps * lnw * rsqrt(mean_part(ps^2)+eps); ps [128,n] psum."""
            sqk = wrk.tile([128, n], F32R, tag="sqk", bufs=1)
            nc.scalar.activation(sqk[:, :], ps[:, :], AF.Square)
            ssk = psB.tile([1, n], F32, tag="psB")
            nc.tensor.matmul(ssk[:, :], ones[:, :], sqk[:, :], start=True, stop=True)
            rks = wrk.tile([1, n], F32, tag="rks", bufs=1)
            nc.scalar.activation(rks[0:1, :], ssk[:, :], AF.Sqrt,
                                 bias=epsA[0:1, 0:1], scale=invHD[0:1, 0:1])
            rk = wrk.tile([1, n], F32, tag="rk", bufs=1)
            nc.vector.reciprocal(rk[0:1, :], rks[0:1, :])
            rkb = wrk.tile([128, n], F32, tag="rkb", bufs=1)
            bcast(rk[0:1, :], n, "rkb", rkb)
            nc.vector.scalar_tensor_tensor(out_ap, ps[:, :], lnw[:, 0:1],
                                           rkb[:, :], ALU.mult, ALU.mult)

        def rope(dst, src, cos_t, sin_t, n):
            """dst[0:128,n] f32r from src f32: rows 0..63 roped, 64..127 copy."""
            nc.vector.tensor_copy(dst[ROT:HD, :], src[ROT:HD, :])
            sh = wrk.tile([ROT, n], F32, tag="sh", bufs=1)
            nc.sync.dma_start(out=sh[0:32, :], in_=src[32:64, :])
            nc.sync.dma_start(out=sh[32:64, :], in_=src[0:32, :])
            tm = wrk.tile([ROT, n], F32, tag="tm", bufs=1)
            nc.vector.tensor_tensor(tm[:, :], src[0:ROT, :], cos_t[:, :], ALU.mult)
            tm2 = wrk.tile([ROT, n], F32, tag="tm2", bufs=1)
            nc.vector.tensor_tensor(tm2[:, :], sh[:, :], sin_t[:, :], ALU.mult)
            nc.vector.tensor_tensor(dst[0:ROT, :], tm[:, :], tm2[:, :], ALU.add)

        # ---- k heads: project, rms, rope -> kro[g] [128, T] f32r ----
        kro = []
        for g in range(NKV):
            kr = krop.tile([128, T], BF16, tag="kro")
            for c in range(2):
                sl = slice(c * 512, (c + 1) * 512)
                ps = psA.tile([128, 512], F32, tag="psA")
                for i in range(NDH):
                    wt = w128.tile([128, 128], BF16, tag="w128")
                    nc.sync.dma_start(
                        out=wt[:, :],
                        in_=I["wkT"][i * 128:(i + 1) * 128, g * 128:(g + 1) * 128])
                    nc.tensor.matmul(ps[:, :], wt[:, :], xnT[i][:, sl],
                                     start=(i == 0), stop=(i == NDH - 1))
                kf = wrk.tile([128, 512], F32, tag="kf", bufs=2)
                rms_cols(ps, 512, kln, kf[:, :])
                rope(kr[:, sl], kf, cosf[:, sl], sinf[:, sl], 512)
            kro.append(kr)

        # ---- v token-major [t-tile, 512] f32r ----
        vsb = []
        for j in range(8):
            ps = psA.tile([128, 512], F32, tag="psA")
            for i in range(NDH):
                wt = w512.tile([128, 512], BF16, tag="w512")
                nc.sync.dma_start(out=wt[:, :],
                                  in_=I["wvT"][i * 128:(i + 1) * 128, :])
                nc.tensor.matmul(ps[:, :], xnT[i][:, j * 128:(j + 1) * 128],
                                 wt[:, :], start=(i == 0), stop=(i == NDH - 1))
            vt = vp.tile([128, 512], BF16, tag="vp")
            nc.vector.tensor_copy(vt[:, :], ps[:, :])
            vsb.append(vt)

        # ---- per q-head: project(local), rms, rope, scores, probs, oT ----
        oT = []
        for h in range(NH):
            g = h // (NH // NKV)
            ps = psB.tile([128, TL], F32, tag="psB")
            for i in range(NDH):
                wt = w128.tile([128, 128], BF16, tag="w128")
                nc.sync.dma_start(
                    out=wt[:, :],
                    in_=I["wqT"][i * 128:(i + 1) * 128, h * 128:(h + 1) * 128])
                nc.tensor.matmul(ps[:, :], wt[:, :], xnTl[i][:, :],
                                 start=(i == 0), stop=(i == NDH - 1))
            qf = wrk.tile([128, TL], F32, tag="qf", bufs=2)
            rms_cols(ps, TL, qln, qf[:, :])
            qr = wrk.tile([128, TL], BF16, tag="qr", bufs=2)
            rope(qr, qf, cosl, sinl, TL)
            # scores^T tiles [tk 128, tq 128]; probs = exp(s*SCAL); oT accum
            pso = psB.tile([128, TL], F32, tag="psB")
            psz = psB.tile([1, TL], F32, tag="psB")
            for tk in range(8):
                sps = psB.tile([128, TL], F32, tag="psB")
                nc.tensor.matmul(sps[:, :], kro[g][:, tk * 128:(tk + 1) * 128],
                                 qr[:, :], start=True, stop=True)
                pr = wrk.tile([128, TL], BF16, tag="pr", bufs=3)
                nc.scalar.activation(pr[:, :], sps[:, :], AF.Exp,
                                     scale=scalA[:, 0:1])
                nc.tensor.matmul(pso[:, :], vsb[tk][:, g * 128:(g + 1) * 128],
                                 pr[:, :], start=(tk == 0), stop=(tk == 7))
                nc.tensor.matmul(psz[:, :], ones_bf[:, :], pr[:, :],
                                 start=(tk == 0), stop=(tk == 7))
            zr = wrk.tile([1, TL], F32, tag="zr", bufs=2)
            nc.vector.reciprocal(zr[0:1, :], psz[:, :])
            zbc = wrk.tile([128, TL], F32, tag="zbc", bufs=2)
            bcast(zr[0:1, :], TL, "zbc", zbc)
            ot = otp.tile([128, TL], BF16, tag="oT")
            nc.vector.tensor_tensor(ot[:, :], pso[:, :], zbc[:, :], ALU.mult)
            oT.append(ot)

        # ---- attn_outT + residual -> xT; rms -> hT (f32r) + xout/ag_in ----
        ag_in = dpool.tile([H, TL], F32R, tag="agin")
        ag_out = dpool.tile([NCORES * H, TL], F32R, tag="agout",
                            addr_space="Shared")
        hT_l = []
        for i in range(NDH):
            ps = psB.tile([128, TL], F32, tag="psB")
            for d in range(NH):
                wt = w128.tile([128, 128], BF16, tag="w128")
                nc.sync.dma_start(
                    out=wt[:, :],
                    in_=I["wdT"][d * 128:(d + 1) * 128, i * 128:(i + 1) * 128])
                nc.tensor.matmul(ps[:, :], wt[:, :], oT[d][:, :],
                                 start=(d == 0), stop=(d == NH - 1))
            hl = wrk.tile([128, TL], F32, tag="hl", bufs=2)
            nc.sync.dma_start(out=hl[:, :], in_=I["hidTl"][i * 128:(i + 1) * 128, :])
            xt = agl.tile([128, TL], F32, tag="xT")
            nc.vector.tensor_tensor(xt[:, :], ps[:, :], hl[:, :], ALU.add)
            nc.sync.dma_start(out=xout[i * 128:(i + 1) * 128, :], in_=xt[:, :])
            hT_l.append(xt)
        # second rms (over H, partition dim) via ones-matmul on squares
        ss2 = psB.tile([1, TL], F32, tag="psB")
        sq2t = []
        for i in range(NDH):
            s2 = wrk.tile([128, TL], F32R, tag="s2", bufs=16)
            nc.scalar.activation(s2[:, :], hT_l[i][:, :], AF.Square)
            sq2t.append(s2)
        for i in range(NDH):
            nc.tensor.matmul(ss2[:, :], ones[:, :], sq2t[i][:, :],
                             start=(i == 0), stop=(i == NDH - 1))
        r2s = wrk.tile([1, TL], F32, tag="r2s", bufs=1)
        nc.scalar.activation(r2s[0:1, :], ss2[:, :], AF.Sqrt,
                             bias=epsA[0:1, 0:1], scale=invH[0:1, 0:1])
        r2 = wrk.tile([1, TL], F32, tag="r2", bufs=1)
        nc.vector.reciprocal(r2[0:1, :], r2s[0:1, :])
        r2b = wrk.tile([128, TL], F32, tag="r2b", bufs=1)
        bcast(r2[0:1, :], TL, "r2b", r2b)
        hTt = []
        for i in range(NDH):
            ln2 = wrk.tile([128, 1], F32, tag="ln2", bufs=2)
            nc.sync.dma_start(out=ln2[:, :], in_=I["ln2c"][i * 128:(i + 1) * 128, :])
            ht = agl.tile([128, TL], F32R, tag="hTl")
            nc.vector.scalar_tensor_tensor(ht[:, :], hT_l[i][:, :], ln2[:, 0:1],
                                           r2b[:, :], ALU.mult, ALU.mult)
            nc.sync.dma_start(out=ag_in[i * 128:(i + 1) * 128, :], in_=ht[:, :])
            hTt.append(ht)

        nc.gpsimd.collective_compute(
            "AllGather", ALU.bypass, ins=[ag_in], outs=[ag_out],
            replica_groups=[list(range(NCORES))])

        # ---- load gathered hT [2048, 1024] f32r into big pool ----
        agv = ag_out.rearrange("(b d) t -> d b t", b=NCORES)
        hsb = []
        for i in range(NDH):
            t_ = big.tile([128, T], BF16, tag="big")
            nc.gpsimd.dma_start(out=t_[:, :], in_=agv[i * 128:(i + 1) * 128, :, :])
            hsb.append(t_)
        hbf = []
        for i in range(NDH):
            hb = agl.tile([128, TL], BF16, tag="hbf")
            nc.vector.tensor_copy(hb[:, :], hTt[i][:, :])
            hbf.append(hb)
        _BUILT["ctx"] = dict(nc=nc, tc=tc, I=I, routed=routed, own=own,
                             hsb=hsb, hTt=hTt, ones=ones, ebbc=ebbc,
                             pools=dict(cst=cst, a12p=a12p, wrk=wrk, w128=w128,
                                        w512=w512, yp=yp, psA=psA, psB=psB))
        _moe(nc, tc, I, routed, own, hsb, hbf, agv, ebbc,
             a12p, wrk, w128, w512, yp, psA, psB)
    nc.compile()
    _BUILT["nc"] = nc
    return nc


def _moe(nc, tc, I, routed, own, hsb, hbf, agv, ebbc, a12p, wrk, w128, w512,
         yp, psA, psB):
    # ---- routing (replicated, all tokens): we [128,16] f32 per t-tile ----
    gts = []
    for i in range(NDH):
        gt = wrk.tile([128, E], F32R, tag="gt", bufs=16)
        nc.sync.dma_start(out=gt[:, :], in_=I["gT"][i * 128:(i + 1) * 128, :])
        gts.append(gt)
    we_sb = []
    for j in range(8):
        pl = psB.tile([128, E], F32, tag="psB")
        for i in range(NDH):
            hl_ = wrk.tile([128, 128], F32R, tag="hload", bufs=3)
            nc.sync.dma_start(out=hl_[:, :],
                              in_=agv[i * 128:(i + 1) * 128, j, :])
            nc.tensor.matmul(pl[:, :], hl_[:, :], gts[i][:, :],
                             start=(i == 0), stop=(i == NDH - 1))
        s = wrk.tile([128, E], F32, tag="rs", bufs=2)
        nc.scalar.activation(s[:, :], pl[:, :], AF.Sigmoid)
        sfr = wrk.tile([128, E], F32, tag="sfr", bufs=2)
        nc.vector.tensor_tensor(sfr[:, :], s[:, :], ebbc[:, :], ALU.add)
        msk = wrk.tile([128, E], F32, tag="msk", bufs=2)
        m1 = wrk.tile([128, 2], F32, tag="m1", bufs=2)
        m2 = wrk.tile([128, 2], F32, tag="m2", bufs=2)
        tmp8 = wrk.tile([128, 8], F32, tag="tmp8", bufs=2)
        for g in range(2):
            hv = sfr[:, g * 8:(g + 1) * 8]
            nc.vector.tensor_reduce(m1[:, g:g + 1], hv, mybir.AxisListType.X,
                                    ALU.max)
            eq = wrk.tile([128, 8], F32, tag="eq", bufs=2)
            nc.vector.tensor_scalar(eq[:, :], hv, m1[:, g:g + 1], None,
                                    ALU.is_equal)
            nc.vector.scalar_tensor_tensor(tmp8[:, :], eq[:, :], -1e30,
                                           hv, ALU.mult, ALU.add)
            nc.vector.tensor_reduce(m2[:, g:g + 1], tmp8[:, :],
                                    mybir.AxisListType.X, ALU.max)
        gs = wrk.tile([128, 2], F32, tag="gs", bufs=2)
        nc.vector.tensor_tensor(gs[:, :], m1[:, :], m2[:, :], ALU.add)
        gd = wrk.tile([128, 1], F32, tag="gd", bufs=2)
        nc.vector.tensor_tensor(gd[:, :], gs[:, 0:1], gs[:, 1:2], ALU.subtract)
        ka = wrk.tile([128, 2], F32, tag="ka", bufs=2)
        nc.vector.tensor_scalar(ka[:, 0:1], gd[:, :], 0.0, None, ALU.is_ge)
        nc.vector.tensor_scalar(ka[:, 1:2], ka[:, 0:1], -1.0, 1.0,
                                ALU.mult, ALU.add)
        for g in range(2):
            nc.vector.tensor_scalar(msk[:, g * 8:(g + 1) * 8],
                                    sfr[:, g * 8:(g + 1) * 8],
                                    ka[:, g:g + 1], None, ALU.mult)
        # 4th-largest threshold of msk
        w0 = wrk.tile([128, E], F32, tag="w0", bufs=2)
        nc.vector.tensor_copy(w0[:, :], msk[:, :])
        tau = wrk.tile([128, 1], F32, tag="tau", bufs=2)
        lt = wrk.tile([128, E], F32, tag="lt", bufs=2)
        for it in range(3):
            nc.vector.tensor_reduce(tau[:, :], w0[:, :], mybir.AxisListType.X,
                                    ALU.max)
            nc.vector.tensor_scalar(lt[:, :], w0[:, :], tau[:, 0:1], None,
                                    ALU.is_lt)
            nc.vector.tensor_tensor(w0[:, :], w0[:, :], lt[:, :], ALU.mult)
        nc.vector.tensor_reduce(tau[:, :], w0[:, :], mybir.AxisListType.X,
                                ALU.max)
        sel = wrk.tile([128, E], F32, tag="sel", bufs=2)
        nc.vector.tensor_scalar(sel[:, :], msk[:, :], tau[:, 0:1], None,
                                ALU.is_ge)
        wsel = wrk.tile([128, E], F32, tag="wsel", bufs=2)
        nc.vector.tensor_tensor(wsel[:, :], s[:, :], sel[:, :], ALU.mult)
        dn = wrk.tile([128, 1], F32, tag="dn", bufs=2)
        nc.vector.tensor_reduce(dn[:, :], wsel[:, :], mybir.AxisListType.X,
                                ALU.add)
        nc.vector.tensor_scalar(dn[:, :], dn[:, :], 1e-20, None, ALU.add)
        rc = wrk.tile([128, 1], F32, tag="rc", bufs=2)
        nc.vector.reciprocal(rc[:, :], dn[:, :])
        we = wrk.tile([128, E], F32, tag="we", bufs=16)
        nc.vector.tensor_scalar(we[:, :], wsel[:, :], rc[:, 0:1], None,
                                ALU.mult)
        we_sb.append(we)

    # ---- routed experts: dense over all tokens, 2 local experts ----
    for tc_i in range(2):          # token chunk of 512
        tsl = slice(tc_i * 512, (tc_i + 1) * 512)
        a12 = {}
        for mi in range(NMI):
            pg0 = psA.tile([128, 512], F32, tag="psA")
            pu0 = psA.tile([128, 512], F32, tag="psA")
            pg1 = psA.tile([128, 512], F32, tag="psA")
            pu1 = psA.tile([128, 512], F32, tag="psA")
            for i in range(NDH):
                rh = hsb[i][:, tsl]
                for (wn, ps) in (("g0T", pg0), ("u0T", pu0),
                                 ("g1T", pg1), ("u1T", pu1)):
                    wt = w128.tile([128, 128], BF16, tag="w128")
                    nc.sync.dma_start(
                        out=wt[:, :],
                        in_=I[wn][i * 128:(i + 1) * 128, mi * 128:(mi + 1) * 128])
                    nc.tensor.matmul(ps[:, :], wt[:, :], rh,
                                     start=(i == 0), stop=(i == NDH - 1))
            for e, (pg, pu) in enumerate(((pg0, pu0), (pg1, pu1))):
                sg = wrk.tile([128, 512], F32, tag="sg", bufs=3)
                nc.scalar.activation(sg[:, :], pg[:, :], AF.Silu)
                at = a12p.tile([128, 512], BF16, tag="a12")
                nc.vector.tensor_tensor(at[:, :], sg[:, :], pu[:, :], ALU.mult)
                a12[(e, mi)] = at
        for ho in range(4):
            ed0 = []
            ed1 = []
            for mi in range(NMI):
                w0_ = w512.tile([128, 512], BF16, tag="edp0", bufs=8)
                nc.sync.dma_start(
                    out=w0_[:, :],
                    in_=I["d0T"][mi * 128:(mi + 1) * 128, ho * 512:(ho + 1) * 512])
                ed0.append(w0_)
                w1_ = w512.tile([128, 512], BF16, tag="edp1", bufs=8)
                nc.sync.dma_start(
                    out=w1_[:, :],
                    in_=I["d1T"][mi * 128:(mi + 1) * 128, ho * 512:(ho + 1) * 512])
                ed1.append(w1_)
            for ts in range(4):
                jj = tc_i * 4 + ts
                cs = slice(ts * 128, (ts + 1) * 128)
                p0 = psA.tile([128, 512], F32, tag="psA")
                for mi in range(NMI):
                    nc.tensor.matmul(p0[:, :], a12[(0, mi)][:, cs], ed0[mi][:, :],
                                     start=(mi == 0), stop=(mi == NMI - 1))
                y = yp.tile([128, 512], F32, tag="y")
                nc.vector.tensor_scalar(y[:, :], p0[:, :],
                                        we_sb[jj][:, 0:1], None, ALU.mult)
                p1 = psA.tile([128, 512], F32, tag="psA")
                for mi in range(NMI):
                    nc.tensor.matmul(p1[:, :], a12[(1, mi)][:, cs], ed1[mi][:, :],
                                     start=(mi == 0), stop=(mi == NMI - 1))
                nc.vector.scalar_tensor_tensor(y[:, :], p1[:, :],
                                               we_sb[jj][:, 1:2], y[:, :],
                                               ALU.mult, ALU.add)
                nc.sync.dma_start(
                    out=routed[jj * 128:(jj + 1) * 128, ho * 512:(ho + 1) * 512],
                    in_=y[:, :])

    # ---- shared expert on local 128 tokens ----
    a12s = []
    for mi in range(NMI):
        pg = psA.tile([128, TL], F32, tag="psA")
        pu = psA.tile([128, TL], F32, tag="psA")
        for i in range(NDH):
            for (wn, ps) in (("sgT", pg), ("suT", pu)):
                wt = w128.tile([128, 128], BF16, tag="w128")
                nc.sync.dma_start(
                    out=wt[:, :],
                    in_=I[wn][i * 128:(i + 1) * 128, mi * 128:(mi + 1) * 128])
                nc.tensor.matmul(ps[:, :], wt[:, :], hbf[i][:, :],
                                 start=(i == 0), stop=(i == NDH - 1))
        sg = wrk.tile([128, TL], F32, tag="sgs", bufs=2)
        nc.scalar.activation(sg[:, :], pg[:, :], AF.Silu)
        at = a12p.tile([128, TL], BF16, tag="a12s", bufs=8)
        nc.vector.tensor_tensor(at[:, :], sg[:, :], pu[:, :], ALU.mult)
        a12s.append(at)
    for ho in range(4):
        ps = psA.tile([128, 512], F32, tag="psA")
        for mi in range(NMI):
            wt = w512.tile([128, 512], BF16, tag="sdw", bufs=4)
            nc.sync.dma_start(
                out=wt[:, :],
                in_=I["sdT"][mi * 128:(mi + 1) * 128, ho * 512:(ho + 1) * 512])
            nc.tensor.matmul(ps[:, :], a12s[mi][:, :], wt[:, :],
                             start=(mi == 0), stop=(mi == NMI - 1))
        y = yp.tile([128, 512], F32, tag="y")
        nc.vector.tensor_copy(y[:, :], ps[:, :])
        nc.sync.dma_start(out=own[:, ho * 512:(ho + 1) * 512], in_=y[:, :])


def _prep(inputs):
    """Per-core input dicts from full inputs."""
    hs = np.ascontiguousarray(inputs["hidden_states"][0])      # [T, H]
    hidT = np.ascontiguousarray(hs.T)                          # [H, T]
    cos = np.ascontiguousarray(inputs["cos"][0].T)             # [ROT, T]
    sin = inputs["sin"][0].T                                   # [ROT, T]
    sgn = np.ones((ROT, 1), np.float32)
    sgn[:32] = -1.0
    sins = np.ascontiguousarray(sin * sgn)
    wqkv = inputs["w_qkv"]
    wqT = np.ascontiguousarray(wqkv[:NH * HD].T)
    wkT = np.ascontiguousarray(wqkv[NH * HD:NH * HD + NKV * HD].T)
    wvT = np.ascontiguousarray(wqkv[NH * HD + NKV * HD:].T)
    wdT = np.ascontiguousarray(inputs["w_dense"].T)
    maps = []
    for c in range(NCORES):
        glo = c // 4
        loc = [2 * c, 2 * c + 1]
        grp = [glo * 8 + k for k in range(8)]
        rest = [e for e in grp if e not in loc]
        other = [(1 - glo) * 8 + k for k in range(8)]
        perm = loc + rest + other
        m = dict(
            hidT=hidT, hidTl=np.ascontiguousarray(hidT[:, c * TL:(c + 1) * TL]),
            onec=np.ones((128, 1), np.float32),
            cosl=np.ascontiguousarray(cos[:, c * TL:(c + 1) * TL]),
            sinl=np.ascontiguousarray(sins[:, c * TL:(c + 1) * TL]),
            cosf=cos, sinf=sins,
            qln=np.ascontiguousarray(inputs["q_ln_w"][:, None]),
            kln=np.ascontiguousarray(inputs["k_ln_w"][:, None]),
            ln1c=np.ascontiguousarray(inputs["ln1_w"][:, None]),
            ln2c=np.ascontiguousarray(inputs["ln2_w"][:, None]),
            wqT=wqT, wkT=wkT, wvT=wvT, wdT=wdT,
            gT=np.ascontiguousarray(inputs["gate_w"][perm].T),
            eb=np.ascontiguousarray(inputs["expert_bias"][perm][None, :]),
            g0T=np.ascontiguousarray(inputs["eg"][loc[0]].T),
            u0T=np.ascontiguousarray(inputs["eu"][loc[0]].T),
            d0T=np.ascontiguousarray(inputs["ed"][loc[0]].T),
            g1T=np.ascontiguousarray(inputs["eg"][loc[1]].T),
            u1T=np.ascontiguousarray(inputs["eu"][loc[1]].T),
            d1T=np.ascontiguousarray(inputs["ed"][loc[1]].T),
            sgT=np.ascontiguousarray(inputs["sg"].T),
            suT=np.ascontiguousarray(inputs["su"].T),
            sdT=np.ascontiguousarray(inputs["sd"].T),
        )
        import ml_dtypes
        bfk = {"wqT", "wkT", "wvT", "wdT", "g0T", "u0T", "d0T", "g1T", "u1T",
               "d1T", "sgT", "suT", "sdT"}
        maps.append({k: (np.asarray(v, ml_dtypes.bfloat16) if k in bfk
                         else np.asarray(v, np.float32)) for k, v in m.items()})
    return maps


def kernel(**inputs):
    nc = _build()
    maps = _prep(inputs)
    res = run_bass_kernel_spmd(nc, maps, list(range(NCORES)),
                               **_BUILT.get("runkw", {}))
    _BUILT["res"] = res
    out = np.zeros((T, H), np.float32)
    for c in range(NCORES):
        r = res.results[c]
        out += r["routed"]
        out[c * TL:(c + 1) * TL] += r["own"] + r["xout"].T
    return out.reshape(B, S, H)



# revision 13
# speedup vs baseline: 2.3769x; 2.3769x over previous
"""LLaDA2 MoE decoder layer on 8 TRN2 NeuronCores — sparse MoE version.

Token-sharded attention (each core: all 16 heads for its 128 tokens, kv
projection replicated).  q/k are RMS-normalized after projection, so the
pre-attention per-token RMS factor cancels for them and is applied only
to V (per-token column scale).  Output projection produces token-major
x2; rms2 along the free dim.  One AllGather of [128, 2064] bf16 payload
per core (2048 normed-hidden cols + 16 routing-weight cols).
Expert-parallel sparse MoE: each core owns 2 experts; on-device
compaction (triangular-matmul cumsum + indirect scatter of token ids),
indirect row-gather of selected tokens, capacity 320/expert.  Host
scatter-adds per-expert outputs and adds shared expert + attention
residual for local tokens.
"""
import numpy as np
import concourse.bass as bass
import concourse.bacc as bacc
import concourse.mybir as mybir
import concourse.tile as tile
from concourse.bass_utils import run_bass_kernel_spmd

AF = mybir.ActivationFunctionType
ALU = mybir.AluOpType
F32 = mybir.dt.float32
F32R = mybir.dt.float32r
BF16 = mybir.dt.bfloat16
I32 = mybir.dt.int32

B, S, H = 1, 1024, 2048
NH, HD, NKV, ROT = 16, 128, 4, 64
E, TOPK, G = 16, 4, 2
MI = 1024
T = S
NCORES = 8
TL = T // NCORES
SCAL = HD ** -0.5
EPS = 1e-6
NDH = H // 128
NMI = MI // 128
EC = 320                      # capacity per expert (3 gathers: 128/128/64)
PK = [128, 128, 64]
W = H + E                     # ag payload width: x cols + we cols
BIGS = 50000.0

_BUILT = {}


def _spec():
    return [
        ("hidT", [H, T], F32), ("hidTl", [H, TL], F32),
        ("hloc", [TL, H], F32),
        ("onec", [128, 1], F32R),
        ("cosl", [ROT, TL], F32), ("sinl", [ROT, TL], F32),
        ("cosf", [ROT, T], F32), ("sinf", [ROT, T], F32),
        ("qln", [HD, 1], F32), ("kln", [HD, 1], F32),
        ("ln1c", [H, 1], F32), ("ln2bc", [128, H], BF16),
        ("wqT", [H, NH * HD], BF16), ("wkvT", [H, 2 * NKV * HD], BF16),
        ("wdT", [NH * HD, H], BF16),
        ("gT", [H, E], BF16), ("eb", [1, E], F32),
        ("eoh0", [128, E], F32), ("eoh1", [128, E], F32),
        ("tok8", [128, 8], F32), ("ut128", [128, 128], BF16),
        ("gu0I", [H, 2 * MI], BF16), ("gu1I", [H, 2 * MI], BF16),
        ("d0T", [MI, H], BF16), ("d1T", [MI, H], BF16),
        ("sguI", [H, 2 * MI], BF16), ("sdT", [MI, H], BF16),
    ]


def _build():
    if "nc" in _BUILT:
        return _BUILT["nc"]
    nc = bacc.Bacc("TRN2", target_bir_lowering=False, debug=False,
                   num_devices=NCORES)
    I = {}
    for name, shp, dt in _spec():
        I[name] = nc.dram_tensor(name, shp, dt, kind="ExternalInput")
    xout = nc.dram_tensor("xout", [TL, H], F32, kind="ExternalOutput")
    own = nc.dram_tensor("own", [TL, H], BF16, kind="ExternalOutput")
    y0 = nc.dram_tensor("y0", [EC, H], BF16, kind="ExternalOutput")
    y1 = nc.dram_tensor("y1", [EC, H], BF16, kind="ExternalOutput")
    idx0 = nc.dram_tensor("idx0", [EC, 1], F32, kind="ExternalOutput")
    idx1 = nc.dram_tensor("idx1", [EC, 1], F32, kind="ExternalOutput")

    with tile.TileContext(nc) as tc, \
         tc.tile_pool(name="cst", bufs=1) as cst, \
         tc.tile_pool(name="big", bufs=16) as big, \
         tc.tile_pool(name="xnl", bufs=16) as xnl, \
         tc.tile_pool(name="kro", bufs=4) as krop, \
         tc.tile_pool(name="vp", bufs=8) as vp, \
         tc.tile_pool(name="otp", bufs=16) as otp, \
         tc.tile_pool(name="ws", bufs=18) as ws, \
         tc.tile_pool(name="wrk", bufs=2) as wrk, \
         tc.tile_pool(name="pst", bufs=1) as pst, \
         tc.tile_pool(name="xg", bufs=1) as xgp, \
         tc.tile_pool(name="a12", bufs=9) as a12p, \
         tc.tile_pool(name="yp", bufs=2) as yp, \
         tc.tile_pool(name="psA", bufs=4, space="PSUM") as psA, \
         tc.tile_pool(name="psB", bufs=4, space="PSUM") as psB, \
         tc.tile_pool(name="dram", bufs=1, space="DRAM") as dpool:

        ones = cst.tile([128, 1], F32R, tag="ones")
        nc.sync.dma_start(out=ones[:, :], in_=I["onec"][:, :])
        ones_bf = cst.tile([128, 1], BF16, tag="ones_bf")
        nc.vector.memset(ones_bf[:, :], 1.0)
        epsA = cst.tile([128, 1], F32, tag="epsA")
        nc.vector.memset(epsA[:, :], EPS)
        invH = cst.tile([128, 1], F32, tag="invH")
        nc.vector.memset(invH[:, :], 1.0 / H)
        invHD = cst.tile([128, 1], F32, tag="invHD")
        nc.vector.memset(invHD[:, :], 1.0 / HD)
        scalA = cst.tile([128, 1], F32, tag="scalA")
        nc.vector.memset(scalA[:, :], SCAL)

        def cload(name, shp, key, dt=F32):
            t_ = cst.tile(shp, dt, tag=key)
            nc.sync.dma_start(out=t_[:, :], in_=I[name][:, :])
            return t_
        qln = cload("qln", [HD, 1], "qln")
        kln = cload("kln", [HD, 1], "kln")
        cosl = cload("cosl", [ROT, TL], "cosl")
        sinl = cload("sinl", [ROT, TL], "sinl")
        cosf = cload("cosf", [ROT, T], "cosf")
        sinf = cload("sinf", [ROT, T], "sinf")
        ln2bc = cload("ln2bc", [128, H], "ln2bc", BF16)
        eoh0 = cload("eoh0", [128, E], "eoh0")
        eoh1 = cload("eoh1", [128, E], "eoh1")
        tok8f = cload("tok8", [128, 8], "tok8")
        ut128t = cload("ut128", [128, 128], "ut128", BF16)
        ebbc = cst.tile([128, E], F32, tag="ebbc")
        nc.sync.dma_start(out=ebbc[:, :],
                          in_=I["eb"][0:1, :].partition_broadcast(128))
        gtall = cst.tile([128, NDH * E], BF16, tag="gtall")
        gtv = gtall.rearrange("p (b e) -> p b e", b=NDH)
        nc.sync.dma_start(out=gtv[:, :, :],
                          in_=I["gT"][:, :].rearrange("(b p) e -> p b e",
                                                      b=NDH))

        def bcast(row_ap, n, tag, out_tile):
            d_ = dpool.tile([1, n], F32, tag=tag + "_d", bufs=2,
                            name=tag + "_d")
            nc.sync.dma_start(out=d_[0:1, :], in_=row_ap)
            nc.sync.dma_start(out=out_tile[:, :],
                              in_=d_[0:1, :].partition_broadcast(128))

        # ---- single pass over hidT: squares for r_row + xln (x*ln1) ----
        ssq = [psB.tile([1, 512], F32, tag="psB", name=f"ssq{c}")
               for c in range(2)]
        xln = []
        for i in range(NDH):
            lnc = wrk.tile([128, 1], F32, tag="lnc", bufs=2)
            nc.sync.dma_start(out=lnc[:, :], in_=I["ln1c"][i * 128:(i + 1) * 128, :])
            xt = big.tile([128, T], BF16, tag="big")
            for c in range(2):
                ht = wrk.tile([128, 512], F32, tag="hidT", bufs=2)
                nc.sync.dma_start(
                    out=ht[:, :],
                    in_=I["hidT"][i * 128:(i + 1) * 128,
                                  c * 512:(c + 1) * 512])
                sq = wrk.tile([128, 512], BF16, tag="sq", bufs=2)
                nc.scalar.activation(sq[:, :], ht[:, :], AF.Square)
                nc.tensor.matmul(ssq[c][:, :], ones_bf[:, :], sq[:, :],
                                 start=(i == 0), stop=(i == NDH - 1))
                nc.vector.tensor_scalar(xt[:, c * 512:(c + 1) * 512], ht[:, :],
                                        lnc[:, 0:1], None, ALU.mult)
            xln.append(xt)
        rd = dpool.tile([1, T], F32, tag="rd", name="rd")
        for c in range(2):
            rsqc = wrk.tile([1, 512], F32, tag="rsqc", bufs=1)
            nc.scalar.activation(rsqc[0:1, :], ssq[c][:, :],
                                 AF.Sqrt, bias=epsA[0:1, 0:1],
                                 scale=invH[0:1, 0:1])
            rrc = wrk.tile([1, 512], F32, tag="rrc", bufs=1)
            nc.vector.reciprocal(rrc[0:1, :], rsqc[0:1, :])
            nc.sync.dma_start(out=rd[0:1, c * 512:(c + 1) * 512],
                              in_=rrc[0:1, :])
        rcol8 = cst.tile([128, 8], F32, tag="rcol8")
        nc.sync.dma_start(out=rcol8[:, :],
                          in_=rd[0:1, :].rearrange("o (j p) -> p (o j)", p=128))

        # ---- local x*ln1 tiles for q projection (rms-invariant) ----
        xlnl = []
        for i in range(NDH):
            htl = wrk.tile([128, TL], F32, tag="htl", bufs=2)
            nc.sync.dma_start(out=htl[:, :],
                              in_=I["hidTl"][i * 128:(i + 1) * 128, :])
            lnc2 = wrk.tile([128, 1], F32, tag="lnc", bufs=2)
            nc.sync.dma_start(out=lnc2[:, :], in_=I["ln1c"][i * 128:(i + 1) * 128, :])
            xl = xnl.tile([128, TL], BF16, tag="xnTl")
            nc.vector.tensor_scalar(xl[:, :], htl[:, :], lnc2[:, 0:1], None,
                                    ALU.mult)
            xlnl.append(xl)

        def rms_cols(ps, n, lnw, out_ap):
            """out = ps * lnw * rsqrt(mean_part(ps^2)+eps); ps [128,n] psum."""
            sqk = wrk.tile([128, n], F32R, tag="sqk", bufs=1)
            nc.scalar.activation(sqk[:, :], ps[:, :], AF.Square)
            ssk = psB.tile([1, n], F32, tag="psB")
            nc.tensor.matmul(ssk[:, :], ones[:, :], sqk[:, :], start=True, stop=True)
            rks = wrk.tile([1, n], F32, tag="rks", bufs=1)
            nc.scalar.activation(rks[0:1, :], ssk[:, :], AF.Sqrt,
                                 bias=epsA[0:1, 0:1], scale=invHD[0:1, 0:1])
            rk = wrk.tile([1, n], F32, tag="rk", bufs=1)
            nc.vector.reciprocal(rk[0:1, :], rks[0:1, :])
            rkb = wrk.tile([128, n], F32, tag="rkb", bufs=1)
            bcast(rk[0:1, :], n, "rkb", rkb)
            nc.vector.scalar_tensor_tensor(out_ap, ps[:, :], lnw[:, 0:1],
                                           rkb[:, :], ALU.mult, ALU.mult)

        def rope(dst, src, cos_t, sin_t, n):
            """dst[0:128,n] from src f32: rows 0..63 roped, 64..127 copy."""
            nc.vector.tensor_copy(dst[ROT:HD, :], src[ROT:HD, :])
            sh = wrk.tile([ROT, n], F32, tag="sh", bufs=1)
            nc.sync.dma_start(out=sh[0:32, :], in_=src[32:64, :])
            nc.sync.dma_start(out=sh[32:64, :], in_=src[0:32, :])
            tm = wrk.tile([ROT, n], F32, tag="tm", bufs=1)
            nc.vector.tensor_tensor(tm[:, :], src[0:ROT, :], cos_t[:, :], ALU.mult)
            tm2 = wrk.tile([ROT, n], F32, tag="tm2", bufs=1)
            nc.vector.tensor_tensor(tm2[:, :], sh[:, :], sin_t[:, :], ALU.mult)
            nc.vector.tensor_tensor(dst[0:ROT, :], tm[:, :], tm2[:, :], ALU.add)

        # ---- k/v weights staged: wkvT [H, 1024] -> 16 x [128, 1024] ----
        wkv = []
        for i in range(NDH):
            wt = ws.tile([128, 1024], BF16, tag="ws", name=f"wkv{i}")
            nc.sync.dma_start(out=wt[:, :],
                              in_=I["wkvT"][i * 128:(i + 1) * 128, :])
            wkv.append(wt)

        # ---- k heads: project, rms, rope -> kro[g] [128, T] bf16 ----
        kro = []
        for g in range(NKV):
            kr = krop.tile([128, T], BF16, tag="kro")
            for c in range(2):
                sl = slice(c * 512, (c + 1) * 512)
                ps = psA.tile([128, 512], F32, tag="psA")
                for i in range(NDH):
                    nc.tensor.matmul(ps[:, :], wkv[i][:, g * 128:(g + 1) * 128],
                                     xln[i][:, sl],
                                     start=(i == 0), stop=(i == NDH - 1))
                kf = wrk.tile([128, 512], F32, tag="kf", bufs=2)
                rms_cols(ps, 512, kln, kf[:, :])
                rope(kr[:, sl], kf, cosf[:, sl], sinf[:, sl], 512)
            kro.append(kr)

        # ---- v token-major [t-tile, 512], scaled by r per token ----
        vsb = []
        for j in range(8):
            ps = psA.tile([128, 512], F32, tag="psA")
            for i in range(NDH):
                nc.tensor.matmul(ps[:, :], xln[i][:, j * 128:(j + 1) * 128],
                                 wkv[i][:, NKV * HD:], start=(i == 0),
                                 stop=(i == NDH - 1))
            vt = vp.tile([128, 512], BF16, tag="vp")
            nc.vector.tensor_scalar(vt[:, :], ps[:, :], rcol8[:, j:j + 1],
                                    None, ALU.mult)
            vsb.append(vt)

        # ---- per q-head: project(local), rms, rope, scores, probs, oT ----
        oT = []
        for half in range(2):
            wqh = []
            for i in range(NDH):
                wt = ws.tile([128, 1024], BF16, tag="ws", name=f"wq{half}_{i}")
                nc.sync.dma_start(
                    out=wt[:, :],
                    in_=I["wqT"][i * 128:(i + 1) * 128,
                                 half * 1024:(half + 1) * 1024])
                wqh.append(wt)
            for h8 in range(8):
                h = half * 8 + h8
                g = h // (NH // NKV)
                ps = psB.tile([128, TL], F32, tag="psB")
                for i in range(NDH):
                    nc.tensor.matmul(ps[:, :], wqh[i][:, h8 * 128:(h8 + 1) * 128],
                                     xlnl[i][:, :],
                                     start=(i == 0), stop=(i == NDH - 1))
                qf = wrk.tile([128, TL], F32, tag="qf", bufs=2)
                rms_cols(ps, TL, qln, qf[:, :])
                qr = wrk.tile([128, TL], BF16, tag="qr", bufs=2)
                rope(qr, qf, cosl, sinl, TL)
                pso = psB.tile([128, TL], F32, tag="psB")
                psz = psB.tile([1, TL], F32, tag="psB")
                for tk in range(8):
                    sps = psB.tile([128, TL], F32, tag="psB")
                    nc.tensor.matmul(sps[:, :], kro[g][:, tk * 128:(tk + 1) * 128],
                                     qr[:, :], start=True, stop=True)
                    pr = wrk.tile([128, TL], BF16, tag="pr", bufs=3)
                    nc.scalar.activation(pr[:, :], sps[:, :], AF.Exp,
                                         scale=scalA[:, 0:1])
                    nc.tensor.matmul(pso[:, :], vsb[tk][:, g * 128:(g + 1) * 128],
                                     pr[:, :], start=(tk == 0), stop=(tk == 7))
                    nc.tensor.matmul(psz[:, :], ones_bf[:, :], pr[:, :],
                                     start=(tk == 0), stop=(tk == 7))
                zr = wrk.tile([1, TL], F32, tag="zr", bufs=2)
                nc.vector.reciprocal(zr[0:1, :], psz[:, :])
                zbc = wrk.tile([128, TL], F32, tag="zbc", bufs=2)
                bcast(zr[0:1, :], TL, "zbc", zbc)
                ot = otp.tile([128, TL], BF16, tag="oT")
                nc.vector.tensor_tensor(ot[:, :], pso[:, :], zbc[:, :], ALU.mult)
                oT.append(ot)

        # ---- out proj (token-major) + residual -> xtok [TL, H] ----
        pxc = [psA.tile([128, 512], F32, tag="psA", name=f"pxc{c}")
               for c in range(4)]
        for d in range(NH):
            wda = ws.tile([128, 1024], BF16, tag="ws", name=f"wda{d}")
            nc.sync.dma_start(out=wda[:, :],
                              in_=I["wdT"][d * 128:(d + 1) * 128, 0:1024])
            wdb = ws.tile([128, 1024], BF16, tag="ws", name=f"wdb{d}")
            nc.sync.dma_start(out=wdb[:, :],
                              in_=I["wdT"][d * 128:(d + 1) * 128, 1024:2048])
            for c in range(4):
                wsrc = wda if c < 2 else wdb
                nc.tensor.matmul(pxc[c][:, :], oT[d][:, :],
                                 wsrc[:, (c % 2) * 512:(c % 2 + 1) * 512],
                                 start=(d == 0), stop=(d == NH - 1))
        xtok = pst.tile([128, H], F32, tag="xtok")
        for c in range(4):
            hl = wrk.tile([128, 512], F32, tag="hl", bufs=2)
            nc.sync.dma_start(out=hl[:, :],
                              in_=I["hloc"][:, c * 512:(c + 1) * 512])
            nc.vector.tensor_tensor(xtok[:, c * 512:(c + 1) * 512],
                                    pxc[c][:, :], hl[:, :], ALU.add)
        nc.sync.dma_start(out=xout[:, :], in_=xtok[:, :])

        # ---- rms2 along free dim (chunked); payload xtk [128, W] bf16 ----
        ss2p = wrk.tile([128, 4], F32, tag="ss2p", bufs=1)
        for c in range(4):
            sq2 = wrk.tile([128, 512], F32, tag="sq2", bufs=2)
            nc.scalar.activation(sq2[:, :], xtok[:, c * 512:(c + 1) * 512],
                                 AF.Square)
            nc.vector.tensor_reduce(ss2p[:, c:c + 1], sq2[:, :],
                                    mybir.AxisListType.X, ALU.add)
        ss2 = wrk.tile([128, 1], F32, tag="ss2", bufs=1)
        nc.vector.tensor_reduce(ss2[:, :], ss2p[:, :], mybir.AxisListType.X,
                                ALU.add)
        r2s = wrk.tile([128, 1], F32, tag="r2s", bufs=1)
        nc.scalar.activation(r2s[:, :], ss2[:, :], AF.Sqrt,
                             bias=epsA[:, 0:1], scale=invH[:, 0:1])
        r2 = wrk.tile([128, 1], F32, tag="r2", bufs=1)
        nc.vector.reciprocal(r2[:, :], r2s[:, :])
        xtk = pst.tile([128, W], BF16, tag="xtk")
        nc.vector.scalar_tensor_tensor(xtk[:, 0:H], xtok[:, :], r2[:, 0:1],
                                       ln2bc[:, :], ALU.mult, ALU.mult)

        # ---- hTbig: one-shot transpose back to h-major blocks ----
        hTbig = pst.tile([128, H], BF16, tag="hTbig")
        hTv = hTbig.rearrange("p (b t) -> p b t", b=NDH)
        nc.sync.dma_start(out=hTv[:, :, :], in_=xtk[:, 0:H], transpose=True)

        # ---- routing (local tokens) ----
        pl = psB.tile([128, E], F32, tag="psB")
        for b in range(NDH):
            nc.tensor.matmul(pl[:, :], hTbig[:, b * 128:(b + 1) * 128],
                             gtv[:, b, :], start=(b == 0), stop=(b == NDH - 1))
        s = wrk.tile([128, E], F32, tag="rs", bufs=1)
        nc.scalar.activation(s[:, :], pl[:, :], AF.Sigmoid)
        sfr = wrk.tile([128, E], F32, tag="sfr", bufs=1)
        nc.vector.tensor_tensor(sfr[:, :], s[:, :], ebbc[:, :], ALU.add)
        msk = wrk.tile([128, E], F32, tag="msk", bufs=1)
        m1 = wrk.tile([128, 2], F32, tag="m1", bufs=1)
        m2 = wrk.tile([128, 2], F32, tag="m2", bufs=1)
        tmp8 = wrk.tile([128, 8], F32, tag="tmp8", bufs=1)
        for g in range(2):
            hv = sfr[:, g * 8:(g + 1) * 8]
            nc.vector.tensor_reduce(m1[:, g:g + 1], hv, mybir.AxisListType.X,
                                    ALU.max)
            eq = wrk.tile([128, 8], F32, tag="eq", bufs=1)
            nc.vector.tensor_scalar(eq[:, :], hv, m1[:, g:g + 1], None,
                                    ALU.is_equal)
            nc.vector.scalar_tensor_tensor(tmp8[:, :], eq[:, :], -1e30,
                                           hv, ALU.mult, ALU.add)
            nc.vector.tensor_reduce(m2[:, g:g + 1], tmp8[:, :],
                                    mybir.AxisListType.X, ALU.max)
        gs = wrk.tile([128, 2], F32, tag="gs", bufs=1)
        nc.vector.tensor_tensor(gs[:, :], m1[:, :], m2[:, :], ALU.add)
        gd = wrk.tile([128, 1], F32, tag="gd", bufs=1)
        nc.vector.tensor_tensor(gd[:, :], gs[:, 0:1], gs[:, 1:2], ALU.subtract)
        ka = wrk.tile([128, 2], F32, tag="ka", bufs=1)
        nc.vector.tensor_scalar(ka[:, 0:1], gd[:, :], 0.0, None, ALU.is_ge)
        nc.vector.tensor_scalar(ka[:, 1:2], ka[:, 0:1], -1.0, 1.0,
                                ALU.mult, ALU.add)
        for g in range(2):
            nc.vector.tensor_scalar(msk[:, g * 8:(g + 1) * 8],
                                    sfr[:, g * 8:(g + 1) * 8],
                                    ka[:, g:g + 1], None, ALU.mult)
        w0 = wrk.tile([128, E], F32, tag="w0", bufs=1)
        nc.vector.tensor_copy(w0[:, :], msk[:, :])
        tau = wrk.tile([128, 1], F32, tag="tau", bufs=1)
        lt = wrk.tile([128, E], F32, tag="lt", bufs=1)
        for it in range(3):
            nc.vector.tensor_reduce(tau[:, :], w0[:, :], mybir.AxisListType.X,
                                    ALU.max)
            nc.vector.tensor_scalar(lt[:, :], w0[:, :], tau[:, 0:1], None,
                                    ALU.is_lt)
            nc.vector.tensor_tensor(w0[:, :], w0[:, :], lt[:, :], ALU.mult)
        nc.vector.tensor_reduce(tau[:, :], w0[:, :], mybir.AxisListType.X,
                                ALU.max)
        sel = wrk.tile([128, E], F32, tag="sel", bufs=1)
        nc.vector.tensor_scalar(sel[:, :], msk[:, :], tau[:, 0:1], None,
                                ALU.is_ge)
        wsel = wrk.tile([128, E], F32, tag="wsel", bufs=1)
        nc.vector.tensor_tensor(wsel[:, :], s[:, :], sel[:, :], ALU.mult)
        dn = wrk.tile([128, 1], F32, tag="dn", bufs=1)
        nc.vector.tensor_reduce(dn[:, :], wsel[:, :], mybir.AxisListType.X,
                                ALU.add)
        nc.vector.tensor_scalar(dn[:, :], dn[:, :], 1e-20, None, ALU.add)
        rc = wrk.tile([128, 1], F32, tag="rc", bufs=1)
        nc.vector.reciprocal(rc[:, :], dn[:, :])
        nc.vector.tensor_scalar(xtk[:, H:W], wsel[:, :], rc[:, 0:1], None,
                                ALU.mult)

        # ---- AllGather payload ----
        ag_in = dpool.tile([128, W], BF16, tag="agin")
        ag_out = dpool.tile([T, W], BF16, tag="agout", addr_space="Shared")
        nc.sync.dma_start(out=ag_in[:, :], in_=xtk[:, :])
        nc.gpsimd.collective_compute(
            "AllGather", ALU.bypass, ins=[ag_in], outs=[ag_out],
            replica_groups=[list(range(NCORES))])

        # ---- shared expert on local tokens (overlaps collective) ----
        a12s = pst.tile([128, MI], BF16, tag="a12s")
        for jh in range(2):
            sgu = []
            for i in range(NDH):
                wt = ws.tile([128, 1024], BF16, tag="ws", name=f"sgu{jh}_{i}")
                nc.sync.dma_start(
                    out=wt[:, :],
                    in_=I["sguI"][i * 128:(i + 1) * 128,
                                  jh * 1024:(jh + 1) * 1024])
                sgu.append(wt)
            psg = psA.tile([128, 512], F32, tag="psA")
            psu = psA.tile([128, 512], F32, tag="psA")
            for b in range(NDH):
                st = hTbig[:, b * 128:(b + 1) * 128]
                nc.tensor.matmul(psg[:, :], st, sgu[b][:, 0:512],
                                 start=(b == 0), stop=(b == NDH - 1))
                nc.tensor.matmul(psu[:, :], st, sgu[b][:, 512:1024],
                                 start=(b == 0), stop=(b == NDH - 1))
            sgj = wrk.tile([128, 512], F32, tag="sgs", bufs=1)
            nc.scalar.activation(sgj[:, :], psg[:, :], AF.Silu)
            nc.vector.tensor_tensor(a12s[:, jh * 512:(jh + 1) * 512],
                                    sgj[:, :], psu[:, :], ALU.mult)
        a12sT = pst.tile([128, MI], BF16, tag="a12sT")
        a12sTv = a12sT.rearrange("p (b t) -> p b t", b=NMI)
        nc.sync.dma_start(out=a12sTv[:, :, :], in_=a12s[:, :], transpose=True)
        for ch in range(2):
            sdw = []
            for bmi in range(NMI):
                wt = ws.tile([128, 1024], BF16, tag="ws", name=f"sdw{ch}_{bmi}")
                nc.sync.dma_start(
                    out=wt[:, :],
                    in_=I["sdT"][bmi * 128:(bmi + 1) * 128,
                                 ch * 1024:(ch + 1) * 1024])
                sdw.append(wt)
            for c2 in range(2):
                c = ch * 2 + c2
                pd = psB.tile([128, 512], F32, tag="psB")
                for bmi in range(NMI):
                    nc.tensor.matmul(pd[:, :],
                                     a12sT[:, bmi * 128:(bmi + 1) * 128],
                                     sdw[bmi][:, c2 * 512:(c2 + 1) * 512],
                                     start=(bmi == 0), stop=(bmi == NMI - 1))
                yo = yp.tile([128, 512], BF16, tag="y", bufs=2)
                nc.vector.tensor_copy(yo[:, :], pd[:, :])
                nc.sync.dma_start(out=own[:, c * 512:(c + 1) * 512],
                                  in_=yo[:, :])

        # ---- compaction: per local expert, build gather index lists ----
        wet = []
        for j in range(8):
            wj = wrk.tile([128, E], BF16, tag="wet", bufs=8)
            nc.sync.dma_start(out=wj[:, :],
                              in_=ag_out[j * 128:(j + 1) * 128, H:W])
            wet.append(wj)
        zc = wrk.tile([128, 1], F32, tag="zc", bufs=1)
        nc.vector.memset(zc[:, :], 0.0)
        cntb = []
        for e, (eoh, idx_t) in enumerate(((eoh0, idx0), (eoh1, idx1))):
            mwe8 = wrk.tile([128, 8], F32, tag=f"mwe8_{e}", bufs=1)
            for j in range(8):
                tmp16 = wrk.tile([128, E], F32, tag="tmp16", bufs=2)
                nc.vector.tensor_tensor(tmp16[:, :], wet[j][:, :], eoh[:, :],
                                        ALU.mult)
                nc.vector.tensor_reduce(mwe8[:, j:j + 1], tmp16[:, :],
                                        mybir.AxisListType.X, ALU.add)
            mskb = wrk.tile([128, 8], BF16, tag="mskb", bufs=2)
            nc.vector.tensor_scalar(mskb[:, :], mwe8[:, :], 0.0, None,
                                    ALU.is_gt)
            pc = psB.tile([128, 8], F32, tag="psB")
            nc.tensor.matmul(pc[:, :], ut128t[:, :], mskb[:, :],
                             start=True, stop=True)
            pt = psB.tile([1, 8], F32, tag="psB")
            nc.tensor.matmul(pt[:, :], ones_bf[:, :], mskb[:, :],
                             start=True, stop=True)
            offs = wrk.tile([1, 8], F32, tag=f"offs{e}", bufs=1)
            nc.vector.memset(offs[:, :], 0.0)
            for j in range(1, 8):
                nc.vector.tensor_tensor(offs[0:1, j:j + 1],
                                        offs[0:1, j - 1:j],
                                        pt[0:1, j - 1:j], ALU.add)
            cnt = wrk.tile([1, 1], F32, tag=f"cnt{e}", bufs=1)
            nc.vector.tensor_tensor(cnt[0:1, 0:1], offs[0:1, 7:8],
                                    pt[0:1, 7:8], ALU.add)
            cb = wrk.tile([128, 1], F32, tag=f"cntb{e}", bufs=1)
            bcast(cnt[0:1, 0:1], 1, f"cntb{e}", cb)
            cntb.append(cb)
            offsb = wrk.tile([128, 8], F32, tag=f"offsb{e}", bufs=1)
            bcast(offs[0:1, :], 8, f"offsb{e}", offsb)
            slotA = wrk.tile([128, 8], F32, tag="slotA", bufs=2)
            nc.vector.scalar_tensor_tensor(slotA[:, :], pc[:, :], -1.0,
                                           offsb[:, :], ALU.add, ALU.add)
            mf = wrk.tile([128, 8], F32, tag="mf", bufs=2)
            nc.vector.tensor_scalar(mf[:, :], mwe8[:, :], 0.0, None, ALU.is_gt)
            t1 = wrk.tile([128, 8], F32, tag="t1", bufs=2)
            nc.vector.tensor_scalar(t1[:, :], slotA[:, :], -BIGS, None, ALU.add)
            t2 = wrk.tile([128, 8], F32, tag="t2", bufs=2)
            nc.vector.tensor_tensor(t2[:, :], t1[:, :], mf[:, :], ALU.mult)
            slotf = wrk.tile([128, 8], F32, tag="slotf", bufs=2)
            nc.vector.tensor_scalar(slotf[:, :], t2[:, :], BIGS, None, ALU.add)
            sloti = wrk.tile([128, 8], I32, tag="sloti", bufs=2)
            nc.vector.tensor_copy(sloti[:, :], slotf[:, :])
            for k in range(3):
                nc.sync.dma_start(out=idx_t[k * 128:k * 128 + PK[k], :],
                                  in_=zc[0:PK[k], :])
            for j in range(8):
                nc.gpsimd.indirect_dma_start(
                    out=idx_t[:, :],
                    out_offset=bass.IndirectOffsetOnAxis(
                        ap=sloti[:, j:j + 1], axis=0),
                    in_=tok8f[:, j:j + 1], in_offset=None,
                    bounds_check=EC - 1, oob_is_err=False)

        # ---- per expert: gather tokens, expert MLP, store y ----
        for e, (eoh, idx_t, y_t, guT, dT) in enumerate((
                (eoh0, idx0, y0, "gu0I", "d0T"),
                (eoh1, idx1, y1, "gu1I", "d1T"))):
            xgbig = xgp.tile([128, NDH * EC], BF16, tag="xgbig")
            xgv = xgbig.rearrange("p (b s) -> p b s", b=NDH)
            we_eff = wrk.tile([128, 3], F32, tag=f"weff{e}", bufs=1)
            for k in range(3):
                p = PK[k]
                idxf = wrk.tile([128, 1], F32, tag="idxf", bufs=2)
                nc.sync.dma_start(out=idxf[0:p, :],
                                  in_=idx_t[k * 128:k * 128 + p, :])
                idxi = wrk.tile([128, 1], I32, tag="idxi", bufs=2)
                nc.vector.tensor_copy(idxi[0:p, :], idxf[0:p, :])
                xgt = wrk.tile([128, W], BF16, tag="xgt", bufs=2)
                nc.gpsimd.indirect_dma_start(
                    out=xgt[0:p, :], out_offset=None,
                    in_=ag_out[:, :],
                    in_offset=bass.IndirectOffsetOnAxis(
                        ap=idxi[0:p, 0:1], axis=0))
                nc.sync.dma_start(out=xgv[:, :, k * 128:k * 128 + p],
                                  in_=xgt[0:p, 0:H], transpose=True)
                tmpw = wrk.tile([128, E], F32, tag="tmpw", bufs=2)
                nc.vector.tensor_tensor(tmpw[0:p, :], xgt[0:p, H:W],
                                        eoh[0:p, :], ALU.mult)
                wec = wrk.tile([128, 1], F32, tag="wec", bufs=2)
                nc.vector.tensor_reduce(wec[0:p, :], tmpw[0:p, :],
                                        mybir.AxisListType.X, ALU.add)
                vm = wrk.tile([128, 1], F32, tag="vm", bufs=2)
                nc.vector.tensor_tensor(vm[0:p, :], tok8f[0:p, k:k + 1],
                                        cntb[e][0:p, :], ALU.is_lt)
                nc.vector.tensor_tensor(we_eff[0:p, k:k + 1], wec[0:p, :],
                                        vm[0:p, :], ALU.mult)
            a12 = []
            for mh in range(2):
                guw = []
                for i in range(NDH):
                    wt = ws.tile([128, 1024], BF16, tag="ws",
                                 name=f"gu{e}_{mh}_{i}")
                    nc.sync.dma_start(
                        out=wt[:, :],
                        in_=I[guT][i * 128:(i + 1) * 128,
                                   mh * 1024:(mh + 1) * 1024])
                    guw.append(wt)
                for mi4 in range(4):
                    mi = mh * 4 + mi4
                    pg = psA.tile([128, EC], F32, tag="psA")
                    pu = psA.tile([128, EC], F32, tag="psA")
                    for b in range(NDH):
                        mv = xgbig[:, b * EC:(b + 1) * EC]
                        nc.tensor.matmul(pg[:, :],
                                         guw[b][:, mi4 * 128:(mi4 + 1) * 128],
                                         mv, start=(b == 0), stop=(b == NDH - 1))
                        nc.tensor.matmul(
                            pu[:, :],
                            guw[b][:, 512 + mi4 * 128:512 + (mi4 + 1) * 128],
                            mv, start=(b == 0), stop=(b == NDH - 1))
                    sg_ = wrk.tile([128, EC], F32, tag="sg", bufs=2)
                    nc.scalar.activation(sg_[:, :], pg[:, :], AF.Silu)
                    at = a12p.tile([128, EC], BF16, tag="a12")
                    nc.vector.tensor_tensor(at[:, :], sg_[:, :], pu[:, :],
                                            ALU.mult)
                    a12.append(at)
            for ch in range(2):
                dw = []
                for bmi in range(NMI):
                    wt = ws.tile([128, 1024], BF16, tag="ws",
                                 name=f"d{e}_{ch}_{bmi}")
                    nc.sync.dma_start(
                        out=wt[:, :],
                        in_=I[dT][bmi * 128:(bmi + 1) * 128,
                                  ch * 1024:(ch + 1) * 1024])
                    dw.append(wt)
                for c2 in range(2):
                    c = ch * 2 + c2
                    pds = [psA.tile([128, 512], F32, tag="psA",
                                    name=f"pd{c}_{k}") for k in range(3)]
                    for bmi in range(NMI):
                        for k in range(3):
                            nc.tensor.matmul(
                                pds[k][0:PK[k], :],
                                a12[bmi][:, k * 128:k * 128 + PK[k]],
                                dw[bmi][:, c2 * 512:(c2 + 1) * 512],
                                start=(bmi == 0), stop=(bmi == NMI - 1))
                    for k in range(3):
                        p = PK[k]
                        yt = yp.tile([128, 512], BF16, tag="yt", bufs=2)
                        nc.vector.tensor_scalar(yt[0:p, :], pds[k][0:p, :],
                                                we_eff[0:p, k:k + 1], None,
                                                ALU.mult)
                        nc.sync.dma_start(
                            out=y_t[k * 128:k * 128 + p,
                                    c * 512:(c + 1) * 512],
                            in_=yt[0:p, :])
    nc.compile()
    _BUILT["nc"] = nc
    return nc


def _prep(inputs):
    """Per-core input dicts from full inputs."""
    import ml_dtypes
    hs = np.ascontiguousarray(inputs["hidden_states"][0])      # [T, H]
    hidT = np.ascontiguousarray(hs.T)                          # [H, T]
    cos = np.ascontiguousarray(inputs["cos"][0].T)             # [ROT, T]
    sin = inputs["sin"][0].T                                   # [ROT, T]
    sgn = np.ones((ROT, 1), np.float32)
    sgn[:32] = -1.0
    sins = np.ascontiguousarray(sin * sgn)
    wqkv = inputs["w_qkv"]
    wqT = np.ascontiguousarray(wqkv[:NH * HD].T)
    wkvT = np.ascontiguousarray(wqkv[NH * HD:].T)              # [H, 1024]
    wdT = np.ascontiguousarray(inputs["w_dense"].T)
    tok8 = (np.arange(8)[None, :] * 128
            + np.arange(128)[:, None]).astype(np.float32)
    ut128 = np.triu(np.ones((128, 128), np.float32))
    ln2bc = np.broadcast_to(inputs["ln2_w"][None, :], (128, H))

    def interleave_gu(g, u):
        # [H, 2*MI] with blocks [g 0:512 | u 0:512 | g 512: | u 512:]
        gT, uT = np.asarray(g).T, np.asarray(u).T               # [H, MI]
        return np.ascontiguousarray(np.concatenate(
            [gT[:, 0:512], uT[:, 0:512], gT[:, 512:1024], uT[:, 512:1024]],
            axis=1))

    maps = []
    for c in range(NCORES):
        e0, e1 = 2 * c, 2 * c + 1
        eoh0 = np.zeros((128, E), np.float32)
        eoh0[:, e0] = 1.0
        eoh1 = np.zeros((128, E), np.float32)
        eoh1[:, e1] = 1.0
        m = dict(
            hidT=hidT, hidTl=np.ascontiguousarray(hidT[:, c * TL:(c + 1) * TL]),
            hloc=np.ascontiguousarray(hs[c * TL:(c + 1) * TL, :]),
            onec=np.ones((128, 1), np.float32),
            cosl=np.ascontiguousarray(cos[:, c * TL:(c + 1) * TL]),
            sinl=np.ascontiguousarray(sins[:, c * TL:(c + 1) * TL]),
            cosf=cos, sinf=sins,
            qln=np.ascontiguousarray(inputs["q_ln_w"][:, None]),
            kln=np.ascontiguousarray(inputs["k_ln_w"][:, None]),
            ln1c=np.ascontiguousarray(inputs["ln1_w"][:, None]),
            ln2bc=ln2bc,
            wqT=wqT, wkvT=wkvT, wdT=wdT,
            gT=np.ascontiguousarray(inputs["gate_w"].T),
            eb=np.ascontiguousarray(inputs["expert_bias"][None, :]),
            eoh0=eoh0, eoh1=eoh1, tok8=tok8, ut128=ut128,
            gu0I=interleave_gu(inputs["eg"][e0], inputs["eu"][e0]),
            gu1I=interleave_gu(inputs["eg"][e1], inputs["eu"][e1]),
            d0T=np.ascontiguousarray(inputs["ed"][e0].T),
            d1T=np.ascontiguousarray(inputs["ed"][e1].T),
            sguI=interleave_gu(inputs["sg"], inputs["su"]),
            sdT=np.ascontiguousarray(inputs["sd"].T),
        )
        bfk = {"wqT", "wkvT", "wdT", "gu0I", "gu1I", "d0T", "d1T",
               "sguI", "sdT", "ut128", "ln2bc", "gT"}
        maps.append({k: (np.asarray(v, ml_dtypes.bfloat16) if k in bfk
                         else np.asarray(v, np.float32)) for k, v in m.items()})
    return maps


def kernel(**inputs):
    nc = _build()
    maps = _prep(inputs)
    res = run_bass_kernel_spmd(nc, maps, list(range(NCORES)),
                               **_BUILT.get("runkw", {}))
    _BUILT["res"] = res
    out = np.zeros((T, H), np.float32)
    for c in range(NCORES):
        r = res.results[c]
        out[c * TL:(c + 1) * TL] += (np.asarray(r["xout"], np.float32)
                             + np.asarray(r["own"]).astype(np.float32))
        for yk, ik in (("y0", "idx0"), ("y1", "idx1")):
            idx = np.asarray(r[ik]).astype(np.int64).ravel()
            np.add.at(out, idx, np.asarray(r[yk]).astype(np.float32))
    return out.reshape(B, S, H)
